# revision 20
# baseline (speedup 1.0000x reference)
"""Trainium2 Bass kernel for a cross-attention transformer block.

Sharding: 8 cores = 4 batches x 2 query-row halves (pure data parallel,
no collectives). Each core computes the full block for its 1024 query
tokens, duplicating only the K/V projections for the other half's rows.

On-device layout convention:
  - residual stream kept feature-major [C(part), tokens(free)]; fp32
    master copy resident in SBUF (no DRAM round-trips)
  - matmuls in bf16 (fp32 PSUM accumulation)
  - attention q/k LN + RoPE done token-major [tokens(part), 64(free)],
    then PE-transposed to feature-major per head
  - attention o computed FEATURE-major: o^T = v^T p per head with a
    ones column appended to v producing the softmax denominator row;
    normalization via DVE reciprocal + GpSimd partition_broadcast
  - softmax without max subtraction (qk-layernorm bounds |s*scale| ~ 6)
"""

import os
import sys
import contextlib

for _p in ("/opt/trn_rl_repo",):
    if os.path.isdir(_p) and _p not in sys.path:
        sys.path.append(_p)

import numpy as np
import ml_dtypes

import concourse.bass as bass
import concourse.mybir as mybir
import concourse.tile as tile
from concourse import bacc
from concourse.bass_utils import run_bass_kernel_spmd
from concourse.masks import make_identity

BF16 = mybir.dt.bfloat16
F32 = mybir.dt.float32
AF = mybir.ActivationFunctionType
OP = mybir.AluOpType

B, N, M, C, H = 4, 2048, 1024, 1024, 16
HD = C // H            # 64
HID = 4 * C            # 4096
SCALE = 1.0 / np.sqrt(HD)
EPS = 1e-6
NQ = N // 2            # own query tokens per core (1024)
NT = N // 128          # 16 token tiles of full seq
NTQ = NQ // 128        # 8 own token tiles
MT = M // 128          # 8 ctx token tiles
CT = C // 128          # 8 feature tiles
JT = HID // 128        # 32 hidden tiles

_CACHE = {}


def _build_program(flags):
    """Build the single-core Tile program. `flags` controls optional beta
    paths (all-zero betas are skipped)."""
    nc = bacc.Bacc("TRN2", target_bir_lowering=False, debug=False)

    def din(name, shape, dt):
        return nc.dram_tensor(name, list(shape), dt, kind="ExternalInput").ap()

    ffn8_early = not (flags["b1"] or flags["bias_rows"])
    FDT0 = mybir.dt.float8e4 if ffn8_early else BF16
    # --- DRAM inputs (per core) ---
    XT = din("xT", (C, N), FDT0)                 # x[b].T, own rows first
    XOWN = din("x_own", (C, NQ), F32)            # fp32 residual basis
    CTXT = din("ctxT", (C, M), FDT0)
    WQKV = din("wqkv", (C, 3 * C), FDT0)
    SAWO = din("sa_wo", (C, C), FDT0)
    CAWQ = din("ca_wq", (C, C), FDT0)
    CAWK = din("ca_wk", (C, C), FDT0)
    CAWV = din("ca_wv", (C, C), FDT0)
    CAWO = din("ca_wo", (C, C), FDT0)
    ffn8 = not (flags["b1"] or flags["bias_rows"])
    FDT = mybir.dt.float8e4 if ffn8 else BF16
    W1G = din("w1g", (C, HID), FDT)
    W1X = din("w1x", (C, HID), FDT)
    W2 = din("w2", (HID, C), FDT)
    DR = mybir.MatmulPerfMode.DoubleRow if ffn8 else None

    def dr_contract(ps, lhs_t, rhs_t, i_sl, m_sl, nt):
        """Contraction over nt 128-tiles: DoubleRow fp8 (pairs) or bf16."""
        if DR is not None:
            for j in range(nt // 2):
                nc.tensor.matmul(ps, lhs_t[:, 2 * j:2 * j + 2, i_sl],
                                 rhs_t[:, 2 * j:2 * j + 2, m_sl],
                                 start=(j == 0), stop=(j == nt // 2 - 1),
                                 perf_mode=DR)
        else:
            for j in range(nt):
                nc.tensor.matmul(ps, lhs_t[:, j, i_sl], rhs_t[:, j, m_sl],
                                 start=(j == 0), stop=(j == nt - 1))
    FBLOB = din("fblob", (128, 224), F32)        # ls0|ls1|ls2|gate|b1g|b1x|cakg|cakb
    TBLOB = din("tblob", (128, 64, HD), BF16)    # cos/W rope tables, gamma folded
    ROWS = din("rows3", (1, 3 * C), BF16)        # sa_bo|ca_bo|b2
    BQ_SA = din("bq_sa", (NQ, HD), F32) if flags["bq_sa"] else None
    BK_SA = din("bk_sa", (N, HD), F32) if flags["bk_sa"] else None
    BQ_CA = din("bq_ca", (NQ, HD), F32) if flags["bq_ca"] else None

    Y = nc.dram_tensor("y", [C, NQ], F32, kind="ExternalOutput").ap()

    with tile.TileContext(nc) as tc:
        with contextlib.ExitStack() as top:
            consts = top.enter_context(tc.tile_pool(name="consts", bufs=1))

            # ---- constants ----
            ident = consts.tile([128, 128], BF16)
            make_identity(nc, ident[:])
            eps_t = consts.tile([128, 1], F32)
            nc.vector.memset(eps_t[:], EPS)
            ones_row = consts.tile([1, 512], BF16)
            nc.vector.memset(ones_row[:], 1.0)

            fb = consts.tile([128, 224], F32)
            nc.sync.dma_start(fb[:], FBLOB)
            ls0 = fb[:, 0:8]
            ls1 = fb[:, 8:16]
            ls2 = fb[:, 16:24]
            gate = fb[:, 24:32]
            b1g = fb[:, 32:64]
            b1x = fb[:, 64:96]
            cakg = fb[:, 96:160]
            cakb = fb[:, 160:224] if flags["cakb"] else None

            tb = consts.tile([128, 64, HD], BF16)
            nc.sync.dma_start(tb[:], TBLOB)
            cosq_sa = tb[:, 0:8, :]
            wq_sa = tb[:, 8:16, :]
            cosk_sa = tb[:, 16:32, :]
            wk_sa = tb[:, 32:48, :]
            cosq_ca = tb[:, 48:56, :]
            wq_ca = tb[:, 56:64, :]

            if flags["bias_rows"]:
                rows3 = consts.tile([1, 3 * C], BF16)
                nc.sync.dma_start(rows3[:], ROWS)
                sabo = rows3[:, 0:C]
                cabo = rows3[:, C:2 * C]
                b2r = rows3[:, 2 * C:3 * C]
            else:
                sabo = cabo = b2r = None

            def load_tab(ap_in, ntile, tag):
                t = consts.tile([128, ntile, HD], F32, tag=tag)
                nc.sync.dma_start(t[:], ap_in.rearrange("(i p) d -> p i d", p=128))
                return t

            bq_sa = load_tab(BQ_SA, NTQ, "bqsa") if BQ_SA is not None else None
            bk_sa = load_tab(BK_SA, NT, "bksa") if BK_SA is not None else None
            bq_ca = load_tab(BQ_CA, NTQ, "bqca") if BQ_CA is not None else None

            # lt1 = ls1 * tanh(gate)
            th = consts.tile([128, 8], F32)
            nc.scalar.activation(out=th[:], in_=gate, func=AF.Tanh)
            lt1 = consts.tile([128, 8], F32)
            nc.vector.tensor_mul(lt1[:], ls1, th[:])

            # ============ helpers ============
            def _bc_heads(ap2):
                """[128, 64] table -> [128, 8, 64] broadcast view (step-0)."""
                return bass.AP(tensor=ap2.tensor, offset=ap2.offset,
                               ap=[list(ap2.ap[0]), [0, 8], list(ap2.ap[1])])

            def _bc_inner(ap2, n):
                """[128, 8] per-head scalars -> [128, 8, n] broadcast view."""
                return bass.AP(tensor=ap2.tensor, offset=ap2.offset,
                               ap=[list(ap2.ap[0]), list(ap2.ap[1]), [0, n]])

            def _swap512(ap2):
                """[128, 512] -> pair-swapped view [128, 256, 2]."""
                return bass.AP(tensor=ap2.tensor, offset=ap2.offset + 1,
                               ap=[list(ap2.ap[0]), [2, 256], [-1, 2]])

            def ln_rope_chunk(ps, work, trps, heads0, cos_t, w_t, b_t,
                              dest, dest_col, tabi):
                """LN + RoPE on a [128, 512] psum chunk (8 heads), batched,
                writing paired-transposed bf16 into dest[:, jp, dest_col:+128].
                cos_t None => no rope (plain gamma via cakg).
                Engine split: ACT copy/square/sqrt; DVE reduces/normalize/
                recip; GPS stat smalls + rope muls."""
                ps8 = ps[:].rearrange("p (h d) -> p h d", d=HD)
                sq = work.tile([128, 512], BF16, tag="wS")
                nc.scalar.square(sq[:], ps[:])
                sums = work.tile([128, 8], F32, tag="sums")
                nc.vector.tensor_reduce(out=sums[:], in_=ps8,
                                        axis=mybir.AxisListType.X, op=OP.add)
                sum2 = work.tile([128, 8], F32, tag="sum2")
                nc.vector.tensor_reduce(out=sum2[:], in_=sq[:].rearrange(
                    "p (h d) -> p h d", d=HD), axis=mybir.AxisListType.X,
                    op=OP.add)
                mean = work.tile([128, 8], F32, tag="mean")
                nc.vector.tensor_scalar_mul(mean[:], sums[:], 1.0 / HD)
                m2 = work.tile([128, 8], F32, tag="m2")
                nc.vector.tensor_mul(m2[:], mean[:], mean[:])
                var = work.tile([128, 8], F32, tag="var")
                nc.vector.scalar_tensor_tensor(out=var[:], in0=sum2[:],
                                               scalar=1.0 / HD, in1=m2[:],
                                               op0=OP.mult, op1=OP.subtract)
                std = work.tile([128, 8], F32, tag="std")
                nc.scalar.activation(out=std[:], in_=var[:], func=AF.Sqrt,
                                     bias=eps_t[:])
                rstd = work.tile([128, 8], F32, tag="rstd")
                nc.vector.reciprocal_approx_fast(rstd[:], std[:])
                # normalize: qn = (ps - mean_b) * rstd_b   (DVE reads psum)
                xc = work.tile([128, 512], BF16, tag="wX")
                xc8 = xc[:].rearrange("p (h d) -> p h d", d=HD)
                nc.vector.tensor_sub(xc8, ps8, _bc_inner(mean[:], HD))
                qn = work.tile([128, 512], BF16, tag="wQ")
                qn8 = qn[:].rearrange("p (h d) -> p h d", d=HD)
                nc.gpsimd.tensor_mul(qn8, xc8, _bc_inner(rstd[:], HD))
                qr = work.tile([128, 512], BF16, tag="wR")
                qr8 = qr[:].rearrange("p (h d) -> p h d", d=HD)
                if cos_t is not None:
                    t1 = work.tile([128, 512], BF16, tag="wT1")
                    t18 = t1[:].rearrange("p (h d) -> p h d", d=HD)
                    nc.gpsimd.tensor_mul(t18, qn8, _bc_heads(cos_t[:, tabi, :]))
                    t2 = work.tile([128, 512], BF16, tag="wT2")
                    t28 = t2[:].rearrange("p (h d) -> p h d", d=HD)
                    nc.vector.tensor_mul(t28, qn8, _bc_heads(w_t[:, tabi, :]))
                    if b_t is None:
                        nc.gpsimd.tensor_add(
                            qr[:].rearrange("p (a b) -> p a b", b=2),
                            t1[:].rearrange("p (a b) -> p a b", b=2),
                            _swap512(t2[:]))
                    else:
                        t3 = work.tile([128, 512], F32, tag="wC")
                        nc.vector.tensor_add(t3[:].rearrange("p (a b) -> p a b", b=2),
                                             t1[:].rearrange("p (a b) -> p a b", b=2),
                                             _swap512(t2[:]))
                        nc.vector.tensor_add(qr8, t3[:].rearrange(
                            "p (h d) -> p h d", d=HD), _bc_heads(b_t[:, tabi, :]))
                else:
                    # CA k: gamma (+ beta) broadcast over heads
                    if cakb is None:
                        nc.gpsimd.tensor_mul(qr8, qn8, _bc_heads(cakg))
                    else:
                        t3 = work.tile([128, 512], BF16, tag="wC")
                        t38 = t3[:].rearrange("p (h d) -> p h d", d=HD)
                        nc.vector.tensor_mul(t38, qn8, _bc_heads(cakg))
                        nc.vector.tensor_add(qr8, t38, _bc_heads(cakb))
                # paired transposes: [128 t, 128 (dA|dB)] -> [128 d-pair, 128 t]
                trt = trps.tile([128, 512], BF16, tag="trq")
                for jp2 in range(4):
                    nc.tensor.transpose(trt[:, jp2 * 128:(jp2 + 1) * 128],
                                        qr[:, jp2 * 128:(jp2 + 1) * 128],
                                        ident[:])
                jp0 = heads0 // 2
                nc.any.tensor_copy(
                    dest[:, jp0:jp0 + 4, dest_col:dest_col + 128],
                    trt[:].rearrange("p (j t) -> p j t", t=128))

            # fast exp2 constants (Schraudolph, int16 -> bf16 bitcast)
            FE_A = float(SCALE * 128.0 / np.log(2.0))
            FE_B = 16250.5

            def attention(kf_t, v_t, qf_t, of_t, ktiles):
                """Per head pair: s^T = k^T q into a 2-bank psum (row-group
                concurrent), p = exp(scale s), o^T = v_aug^T p feature-major
                with a ones column producing the denominator row, normalize
                via DVE recip + GPS partition_broadcast + DVE mul."""
                with tc.tile_pool(name="att_ps", bufs=2, space="PSUM") as ps_s, \
                     tc.tile_pool(name="att_po", bufs=3, space="PSUM") as ps_o, \
                     tc.tile_pool(name="att_pb", bufs=1, space="PSUM") as ps_b, \
                     tc.tile_pool(name="att_wk", bufs=4) as wk, \
                     tc.tile_pool(name="att_rr", bufs=2) as rrp:
                    for jp in range(CT):
                        hA, hB = 2 * jp, 2 * jp + 1
                        for tqc in range(2):
                            qsl = slice(tqc * 512, (tqc + 1) * 512)
                            o_A = ps_o.tile([128, 512], F32, tag="ops")
                            o_B = ps_o.tile([128, 512], F32, tag="ops")

                            def emit_o(pv, tk):
                                nc.tensor.matmul(
                                    o_A[0:65, :], v_t[:, tk, hA, 0:65],
                                    pv[:, 0, :],
                                    start=(tk == 0), stop=(tk == ktiles - 1))
                                nc.tensor.matmul(
                                    o_B[0:65, :], v_t[:, tk, hB, 0:65],
                                    pv[:, 1, :],
                                    start=(tk == 0), stop=(tk == ktiles - 1))

                            pend = []
                            for tk in range(ktiles):
                                s2 = ps_s.tile([128, 2, 512], F32, tag="sps")
                                nc.tensor.matmul(
                                    s2[:, 0, :], kf_t[0:64, jp, tk * 128:(tk + 1) * 128],
                                    qf_t[0:64, jp, qsl],
                                    start=True, stop=True)
                                nc.tensor.matmul(
                                    s2[:, 1, :], kf_t[64:128, jp, tk * 128:(tk + 1) * 128],
                                    qf_t[64:128, jp, qsl],
                                    start=True, stop=True)
                                if tk % 5 >= 3:
                                    p2i = wk.tile([128, 2, 512], mybir.dt.int16,
                                                  tag="p2i")
                                    nc.vector.tensor_scalar(
                                        out=p2i[:], in0=s2[:], scalar1=FE_A,
                                        scalar2=FE_B, op0=OP.mult, op1=OP.add)
                                    pv = p2i[:].bitcast(BF16)
                                else:
                                    p2 = wk.tile([128, 2, 512], BF16, tag="p2a")
                                    nc.scalar.activation(out=p2[:], in_=s2[:],
                                                         func=AF.Exp, scale=SCALE)
                                    pv = p2[:]
                                pend.append((pv, tk))
                                if len(pend) > 2:
                                    emit_o(*pend.pop(0))
                            for pv, tk in pend:
                                emit_o(pv, tk)
                            # evacuate: normalize columns by denominator row
                            for o_ps, h0 in ((o_A, 0), (o_B, 64)):
                                dr = rrp.tile([1, 512], BF16, tag="dr")
                                nc.scalar.copy(dr[:], o_ps[64:65, :])
                                rb = ps_b.tile([64, 512], F32, tag="rb")
                                nc.tensor.matmul(rb[:], ones_row[0:1, 0:64],
                                                 dr[:], start=True, stop=True)
                                rr = rrp.tile([64, 512], F32, tag="rr")
                                nc.vector.reciprocal_approx_fast(rr[:], rb[:])
                                nc.vector.tensor_mul(
                                    of_t[h0:h0 + 64, jp, qsl],
                                    o_ps[0:64, :], rr[:])

            def project_residual(w_sb_t, act_t, bias_row, scal, xr, out_bf):
                w_sb = w_sb_t[:]
                act_f = act_t[:]
                """xr[:, i, sl] += (w^T act + bias_row) * scal  (in-place on
                the resident fp32 residual); bf16 copy to out_bf."""
                with tc.tile_pool(name="proj_ps", bufs=3, space="PSUM") as pp:
                    for i in range(CT):
                        for tcx in range(2):
                            sl = slice(tcx * 512, (tcx + 1) * 512)
                            ps = pp.tile([128, 512], F32, tag="pp")
                            if bias_row is not None:
                                nc.tensor.matmul(
                                    ps[:], bias_row[0:1, i * 128:(i + 1) * 128],
                                    ones_row[:], start=True, stop=False)
                                for j in range(CT):
                                    nc.tensor.matmul(
                                        ps[:], w_sb[:, j, i * 128:(i + 1) * 128],
                                        act_f[:, j, sl],
                                        start=False, stop=(j == CT - 1))
                            else:
                                dr_contract(ps[:], w_sb_t, act_t,
                                            slice(i * 128, (i + 1) * 128),
                                            sl, CT)
                            nc.vector.scalar_tensor_tensor(
                                out=xr[:, i, sl], in0=ps[:],
                                scalar=scal[:, i:i + 1], in1=xr[:, i, sl],
                                op0=OP.mult, op1=OP.add)
                            if out_bf is not None:
                                nc.gpsimd.tensor_copy(out_bf[:, i, sl],
                                                      xr[:, i, sl])

            # resident residual stream: fp32 master (in-place updated) and
            # bf16 working copy (slot shared between x1_bf and x2_bf)
            resid = top.enter_context(tc.tile_pool(name="resid", bufs=1))
            xbfp = top.enter_context(tc.tile_pool(name="xbfp", bufs=1))
            xr = resid.tile([128, CT, NQ], F32, tag="xr")

            # ================= SA scope =================
            with tc.tile_pool(name="attn_sa", bufs=1) as attn_sa:
                q_f = attn_sa.tile([128, CT, NQ], BF16, tag="qf")
                k_f = attn_sa.tile([128, CT, N], BF16, tag="kf")
                v_sa = attn_sa.tile([128, NT, H, 65], BF16, tag="vsa")
                nc.vector.memset(v_sa[:, :, :, 64:65], 1.0)

                # ---- phase 1: SA qkv + LN/rope + pack (chunk-outer) ----
                with tc.tile_pool(name="p1_x", bufs=1) as p1x, \
                     tc.tile_pool(name="p1_wq", bufs=2) as p1wq, \
                     tc.tile_pool(name="p1_work", bufs=6) as work, \
                     tc.tile_pool(name="p1_ps", bufs=4, space="PSUM") as p1ps, \
                     tc.tile_pool(name="p1_tr", bufs=2, space="PSUM") as p1tr:
                    xT_sb = p1x.tile([128, CT, N], FDT0)
                    nc.gpsimd.dma_start(xT_sb[:],
                                        XT.rearrange("(j p) t -> p j t", p=128))
                    wqkv_r = WQKV.rearrange("(j p) o -> p j o", p=128)
                    for ch in range(6):
                        w_ch = p1wq.tile([128, CT, 512], FDT0, tag="wch")
                        nc.sync.dma_start(w_ch[:],
                                          wqkv_r[:, :, ch * 512:(ch + 1) * 512])
                        ntile = NTQ if ch < 2 else NT
                        for i in range(ntile):
                            ps = p1ps.tile([128, 512], F32, tag="qkv")
                            dr_contract(ps[:], xT_sb,  w_ch,
                                        slice(i * 128, (i + 1) * 128),
                                        slice(0, 512), CT)
                            if ch < 2:       # q
                                ln_rope_chunk(ps, work, p1tr, ch * 8, cosq_sa,
                                              wq_sa, bq_sa, q_f, i * 128, i)
                            elif ch < 4:     # k
                                ln_rope_chunk(ps, work, p1tr, (ch - 2) * 8,
                                              cosk_sa, wk_sa, bk_sa, k_f,
                                              i * 128, i)
                            else:            # v
                                hs = (ch - 4) * 8
                                nc.any.tensor_copy(
                                    v_sa[:, i, hs:hs + 8, 0:64],
                                    ps[:].rearrange("p (h d) -> p h d", d=HD))

                with tc.tile_pool(name="of_sa", bufs=1) as ofp, \
                     tc.tile_pool(name="sawop", bufs=1) as sawop:
                    o_f = ofp.tile([128, CT, NQ], FDT0, tag="of")
                    # prefetch for phase 3 (overlaps attention)
                    nc.gpsimd.dma_start(xr[:],
                                        XOWN.rearrange("(j p) t -> p j t", p=128))
                    wo_sb = sawop.tile([128, CT, C], FDT0, tag="wo")
                    nc.sync.dma_start(wo_sb[:],
                                      SAWO.rearrange("(j p) o -> p j o", p=128))

                    # ---- phase 2: SA attention ----
                    attention(k_f, v_sa, q_f, o_f, NT)

                    # ---- phase 3: SA out proj + residual ----
                    x1_bf = xbfp.tile([128, CT, NQ], FDT0, tag="xbf")
                    project_residual(wo_sb, o_f, sabo, ls0, xr[:], x1_bf[:])

            # ================= CA scope =================
            with tc.tile_pool(name="attn_ca", bufs=1) as attn_ca:
                k_fca = attn_ca.tile([128, CT, M], BF16, tag="kfca")
                v_ca = attn_ca.tile([128, MT, H, 65], BF16, tag="vca")
                nc.vector.memset(v_ca[:, :, :, 64:65], 1.0)
                q_fca = attn_ca.tile([128, CT, NQ], BF16, tag="qfca")

                with tc.tile_pool(name="p4_x", bufs=1) as p4x, \
                     tc.tile_pool(name="p4_w", bufs=2) as p4w, \
                     tc.tile_pool(name="p4_work", bufs=6) as work4, \
                     tc.tile_pool(name="p4_ps", bufs=4, space="PSUM") as p4ps, \
                     tc.tile_pool(name="p4_tr", bufs=2, space="PSUM") as p4tr:
                    ctx_sb = p4x.tile([128, CT, M], FDT0, tag="ctx")
                    nc.gpsimd.dma_start(ctx_sb[:],
                                        CTXT.rearrange("(j p) t -> p j t", p=128))
                    # k then v, chunk-outer
                    for src, is_v in ((CAWK, False), (CAWV, True)):
                        src_r = src.rearrange("(j p) o -> p j o", p=128)
                        for ch in range(2):
                            w_ch = p4w.tile([128, CT, 512], FDT0, tag="wch4")
                            nc.sync.dma_start(w_ch[:],
                                              src_r[:, :, ch * 512:(ch + 1) * 512])
                            for i in range(MT):
                                ps = p4ps.tile([128, 512], F32, tag="kv")
                                dr_contract(ps[:], ctx_sb, w_ch,
                                            slice(i * 128, (i + 1) * 128),
                                            slice(0, 512), CT)
                                if not is_v:
                                    ln_rope_chunk(ps, work4, p4tr, ch * 8, None,
                                                  None, None, k_fca, i * 128, i)
                                else:
                                    hs = ch * 8
                                    nc.any.tensor_copy(
                                        v_ca[:, i, hs:hs + 8, 0:64],
                                        ps[:].rearrange("p (h d) -> p h d", d=HD))
                    # q proj from x1_bf
                    cawq_r = CAWQ.rearrange("(j p) o -> p j o", p=128)
                    for ch in range(2):
                        w_ch = p4w.tile([128, CT, 512], FDT0, tag="wch4")
                        nc.sync.dma_start(w_ch[:],
                                          cawq_r[:, :, ch * 512:(ch + 1) * 512])
                        for i in range(NTQ):
                            ps = p4ps.tile([128, 512], F32, tag="kv")
                            dr_contract(ps[:], x1_bf, w_ch,
                                        slice(i * 128, (i + 1) * 128),
                                        slice(0, 512), CT)
                            ln_rope_chunk(ps, work4, p4tr, ch * 8, cosq_ca,
                                          wq_ca, bq_ca, q_fca, i * 128, i)

                with tc.tile_pool(name="of_ca", bufs=1) as ofcp, \
                     tc.tile_pool(name="cawop", bufs=1) as cawop:
                    o_fca = ofcp.tile([128, CT, NQ], FDT0, tag="ofca")
                    wo2_sb = cawop.tile([128, CT, C], FDT0, tag="wo2")
                    nc.sync.dma_start(wo2_sb[:],
                                      CAWO.rearrange("(j p) o -> p j o", p=128))

                    attention(k_fca, v_ca, q_fca, o_fca, MT)

                    x2_bf = xbfp.tile([128, CT, NQ], FDT, tag="xbf")
                    project_residual(wo2_sb, o_fca, cabo, lt1[:], xr[:],
                                     x2_bf[:])

            # ============ phase 5: SwiGLU FFN ============
            with tc.tile_pool(name="p5_w", bufs=4) as p5w, \
                 tc.tile_pool(name="p5_w2", bufs=2) as p5w2, \
                 tc.tile_pool(name="p5_hp", bufs=1) as p5hp, \
                 tc.tile_pool(name="p5_work", bufs=3) as work5, \
                 tc.tile_pool(name="p5_psg", bufs=2, space="PSUM") as psg, \
                 tc.tile_pool(name="p5_psx", bufs=2, space="PSUM") as psx, \
                 tc.tile_pool(name="p5_psf", bufs=2, space="PSUM") as psf:
                w1g_r = W1G.rearrange("(j p) o -> p j o", p=128)
                w1x_r = W1X.rearrange("(j p) o -> p j o", p=128)
                w2_r = W2.rearrange("(j p) o -> p j o", p=128)
                DR = mybir.MatmulPerfMode.DoubleRow
                for tcx in range(2):
                    sl = slice(tcx * 512, (tcx + 1) * 512)
                    hp = p5hp.tile([128, JT, 512], FDT, tag="hp")
                    for j in range(JT):
                        w1g_j = p5w.tile([128, CT, 128], FDT, tag="w1gj")
                        nc.sync.dma_start(w1g_j[:], w1g_r[:, :, j * 128:(j + 1) * 128])
                        w1x_j = p5w.tile([128, CT, 128], FDT, tag="w1xj")
                        nc.sync.dma_start(w1x_j[:], w1x_r[:, :, j * 128:(j + 1) * 128])
                        g_ps = psg.tile([128, 512], F32, tag="g")
                        x_ps = psx.tile([128, 512], F32, tag="x")
                        if ffn8:
                            for jc in range(CT // 2):
                                nc.tensor.matmul(
                                    g_ps[:], w1g_j[:, 2 * jc:2 * jc + 2, :],
                                    x2_bf[:, 2 * jc:2 * jc + 2, sl],
                                    start=(jc == 0), stop=(jc == CT // 2 - 1),
                                    perf_mode=DR)
                            for jc in range(CT // 2):
                                nc.tensor.matmul(
                                    x_ps[:], w1x_j[:, 2 * jc:2 * jc + 2, :],
                                    x2_bf[:, 2 * jc:2 * jc + 2, sl],
                                    start=(jc == 0), stop=(jc == CT // 2 - 1),
                                    perf_mode=DR)
                        else:
                            for jc in range(CT):
                                nc.tensor.matmul(g_ps[:], w1g_j[:, jc, :],
                                                 x2_bf[:, jc, sl],
                                                 start=(jc == 0), stop=(jc == CT - 1))
                            for jc in range(CT):
                                nc.tensor.matmul(x_ps[:], w1x_j[:, jc, :],
                                                 x2_bf[:, jc, sl],
                                                 start=(jc == 0), stop=(jc == CT - 1))
                        g_sb = work5.tile([128, 512], BF16, tag="gsb")
                        if flags["b1"]:
                            nc.scalar.activation(out=g_sb[:], in_=g_ps[:],
                                                 func=AF.Silu,
                                                 bias=b1g[:, j:j + 1])
                            nc.vector.scalar_tensor_tensor(
                                out=hp[:, j, :], in0=x_ps[:],
                                scalar=b1x[:, j:j + 1],
                                in1=g_sb[:], op0=OP.add, op1=OP.mult)
                        else:
                            # silu(g) = 0.5*g*(1+tanh(g/2)); Tanh is a 1-pass
                            # ACT table func (Silu is ~4x slower). In fp8 mode
                            # g_ps = 64*g, x_ps = 64*xb, h stored as 8*h.
                            gsc = (0.5 / 64.0) if ffn8 else 0.5
                            hsc = (8.0 / (2 * 64.0 * 64.0)) if ffn8 else 0.5
                            nc.scalar.activation(out=g_sb[:], in_=g_ps[:],
                                                 func=AF.Tanh, scale=gsc)
                            u = work5.tile([128, 512], F32, tag="usb")
                            nc.vector.scalar_tensor_tensor(
                                out=u[:], in0=g_sb[:], scalar=1.0,
                                in1=g_ps[:], op0=OP.add, op1=OP.mult)
                            nc.vector.scalar_tensor_tensor(
                                out=hp[:, j, :], in0=u[:], scalar=hsc,
                                in1=x_ps[:], op0=OP.mult, op1=OP.mult)
                    for i in range(CT):
                        w2_i = p5w2.tile([128, JT, 128], FDT, tag="w2i")
                        nc.sync.dma_start(w2_i[:], w2_r[:, :, i * 128:(i + 1) * 128])
                        f_ps = psf.tile([128, 512], F32, tag="f")
                        if b2r is not None:
                            nc.tensor.matmul(
                                f_ps[:], b2r[0:1, i * 128:(i + 1) * 128],
                                ones_row[:], start=True, stop=False)
                        if ffn8:
                            for j in range(JT // 2):
                                nc.tensor.matmul(
                                    f_ps[:], w2_i[:, 2 * j:2 * j + 2, :],
                                    hp[:, 2 * j:2 * j + 2, :],
                                    start=(j == 0), stop=(j == JT // 2 - 1),
                                    perf_mode=DR)
                        else:
                            for j in range(JT):
                                nc.tensor.matmul(f_ps[:], w2_i[:, j, :],
                                                 hp[:, j, :],
                                                 start=(j == 0 and b2r is None),
                                                 stop=(j == JT - 1))
                        y_sb = work5.tile([128, 512], F32, tag="ysb")
                        nc.vector.scalar_tensor_tensor(
                            out=y_sb[:], in0=f_ps[:], scalar=ls2[:, i:i + 1],
                            in1=xr[:, i, sl], op0=OP.mult, op1=OP.add)
                        nc.gpsimd.dma_start(Y[i * 128:(i + 1) * 128, sl], y_sb[:])

    nc.compile()
    return nc


def _rope_tables(rope, g, b):
    """cos/W (swap-multiplier) tables with gamma folded; plus additive beta
    table (or None)."""
    sin, cos = rope[:, :HD], rope[:, HD:]
    W = np.empty_like(sin)
    W[:, 0::2] = sin[:, 1::2]
    W[:, 1::2] = -sin[:, 0::2]
    c1 = (cos * g[None, :]).astype(np.float32)
    w1 = (W * g[None, :]).astype(np.float32)
    bt = None
    if b is not None and np.any(b):
        bw = b[None, :] * W
        bwsw = np.empty_like(bw)
        bwsw[:, 0::2], bwsw[:, 1::2] = bw[:, 1::2], bw[:, 0::2]
        bt = (b[None, :] * cos + bwsw).astype(np.float32)
    return np.ascontiguousarray(c1), np.ascontiguousarray(w1), bt


def _wcast(w, fp8):
    """Weight cast: fp8 e4m3 with x64 scale (absorbed downstream) or bf16."""
    w = np.ascontiguousarray(w)
    if fp8:
        return np.clip(np.asarray(w, np.float32) * 64.0, -240.0, 240.0).astype(
            ml_dtypes.float8_e4m3)
    return w.astype(ml_dtypes.bfloat16)


def _prepare(inputs):
    """Host-side sharding: returns (flags, in_maps) for the 8 cores."""
    f32 = np.float32
    bf = ml_dtypes.bfloat16
    x = np.asarray(inputs["x"], f32)
    ctx = np.asarray(inputs["ctx"], f32)
    rope = np.asarray(inputs["rope"], f32)

    flags = {
        "bq_sa": bool(np.any(inputs["sa_qb"])),
        "bk_sa": bool(np.any(inputs["sa_kb"])),
        "bq_ca": bool(np.any(inputs["ca_qb"])),
        "cakb": bool(np.any(inputs["ca_kb"])),
        "bias_rows": bool(np.any(inputs["sa_bo"]) or np.any(inputs["ca_bo"])
                          or np.any(inputs["b2"])),
        "b1": bool(np.any(inputs["b1g"]) or np.any(inputs["b1x"])),
    }

    def fm(v, nt):  # feature-major [128, nt]
        return np.asarray(v, f32).reshape(nt, 128).T

    ffn8 = not (flags["b1"] or flags["bias_rows"])
    wsc = 4096.0 if ffn8 else 1.0
    fblob = np.zeros((128, 224), f32)
    fblob[:, 0:8] = fm(inputs["ls0"], CT) / wsc
    fblob[:, 8:16] = fm(inputs["ls1"], CT) / wsc
    fblob[:, 16:24] = fm(inputs["ls2"], CT) / (512.0 if ffn8 else 1.0)
    fblob[:, 24:32] = fm(inputs["ca_gate"], CT)
    fblob[:, 32:64] = fm(inputs["b1g"], JT)
    fblob[:, 64:96] = fm(inputs["b1x"], JT)
    fblob[:, 96:160] = np.tile(np.asarray(inputs["ca_kg"], f32)[None, :], (128, 1))
    fblob[:, 160:224] = np.tile(np.asarray(inputs["ca_kb"], f32)[None, :], (128, 1))

    rows3 = np.zeros((1, 3 * C), f32)
    rows3[0, 0:C] = np.asarray(inputs["sa_bo"], f32)
    rows3[0, C:2 * C] = np.asarray(inputs["ca_bo"], f32)
    rows3[0, 2 * C:3 * C] = np.asarray(inputs["b2"], f32)

    shared = {
        "wqkv": _wcast(inputs["wqkv"], ffn8),
        "sa_wo": _wcast(inputs["sa_wo"], ffn8),
        "ca_wq": _wcast(inputs["ca_wq"], ffn8),
        "ca_wk": _wcast(inputs["ca_wk"], ffn8),
        "ca_wv": _wcast(inputs["ca_wv"], ffn8),
        "ca_wo": _wcast(inputs["ca_wo"], ffn8),
        "w1g": _wcast(inputs["w1g"], ffn8),
        "w1x": _wcast(inputs["w1x"], ffn8),
        "w2": _wcast(inputs["w2"], ffn8),
        "fblob": np.ascontiguousarray(fblob),
        "rows3": np.ascontiguousarray(rows3.astype(bf)),
    }

    cq_sa, wq_sa, bq_sa = _rope_tables(rope, np.asarray(inputs["sa_qg"], f32),
                                       np.asarray(inputs["sa_qb"], f32))
    ck_sa, wk_sa, bk_sa = _rope_tables(rope, np.asarray(inputs["sa_kg"], f32),
                                       np.asarray(inputs["sa_kb"], f32))
    cq_ca, wq_ca, bq_ca = _rope_tables(rope, np.asarray(inputs["ca_qg"], f32),
                                       np.asarray(inputs["ca_qb"], f32))

    def tab_fold(t, own_or_perm, ntile):
        # [tok, HD] -> [128, ntile, HD] (token tiles along axis 1)
        sel = t[own_or_perm]
        return sel.reshape(ntile, 128, HD).transpose(1, 0, 2)

    in_maps = []
    for core in range(8):
        b, h = divmod(core, 2)
        own = slice(h * NQ, (h + 1) * NQ)
        oth = slice((1 - h) * NQ, (2 - h) * NQ)
        perm = np.r_[own, oth]
        xp = x[b][perm]                      # [2048, 1024] own rows first
        m = dict(shared)
        adt = ml_dtypes.float8_e4m3 if ffn8 else bf
        m["xT"] = np.ascontiguousarray(xp.T).astype(adt)
        m["x_own"] = np.ascontiguousarray(x[b][own].T)
        m["ctxT"] = np.ascontiguousarray(ctx[b].T).astype(adt)
        tblob = np.zeros((128, 64, HD), f32)
        tblob[:, 0:8] = tab_fold(cq_sa, own, NTQ)
        tblob[:, 8:16] = tab_fold(wq_sa, own, NTQ)
        tblob[:, 16:32] = tab_fold(ck_sa, perm, NT)
        tblob[:, 32:48] = tab_fold(wk_sa, perm, NT)
        tblob[:, 48:56] = tab_fold(cq_ca, own, NTQ)
        tblob[:, 56:64] = tab_fold(wq_ca, own, NTQ)
        m["tblob"] = np.ascontiguousarray(tblob.astype(bf))
        if flags["bq_sa"]:
            m["bq_sa"] = bq_sa[own]
        if flags["bk_sa"]:
            m["bk_sa"] = np.ascontiguousarray(bk_sa[perm])
        if flags["bq_ca"]:
            m["bq_ca"] = bq_ca[own]
        in_maps.append(m)
    return flags, in_maps


def _get_program(flags):
    key = tuple(sorted(flags.items()))
    if key not in _CACHE:
        _CACHE[key] = _build_program(flags)
    return _CACHE[key]


def _run(in_maps, nc, trace=False, trace_kwargs=None):
    return run_bass_kernel_spmd(nc, in_maps, list(range(8)), trace=trace,
                                **(trace_kwargs or {}))


def kernel(**inputs):
    flags, in_maps = _prepare(inputs)
    nc = _get_program(flags)
    res = _run(in_maps, nc)
    out = np.empty((B, N, C), np.float32)
    for core in range(8):
        b, h = divmod(core, 2)
        out[b, h * NQ:(h + 1) * NQ, :] = res.results[core]["y"].T
    return out
